# revision 12
# baseline (speedup 1.0000x reference)
import numpy as np
import ml_dtypes

from concourse import bass, bacc, mybir, tile
from concourse.bass_utils import run_bass_kernel_spmd

F32 = mybir.dt.float32
BF16 = mybir.dt.bfloat16
I32 = mybir.dt.int32
BF = ml_dtypes.bfloat16

T, R, D, H, DK, L = 3, 6, 128, 4, 32, 2
REL_SRC = (0, 1, 2, 0, 1, 2)
REL_DST = (1, 2, 0, 2, 0, 1)
SQRT_DK = float(np.sqrt(DK))
EPS = 1e-5
NCORE = 8
CAP = 256
RELS_OF = [[r for r in range(R) if REL_DST[r] == t] for t in range(T)]
# dst-type processing order per layer, chosen so each edge phase only needs
# the per-type allgathers that have already been issued (overlap-friendly).
ORD = [(2, 0, 1), (1, 2, 0)]

LAST_EXEC_NS = None
LAST_NC = None


def estimate_exec_ns():
    """Cost-model estimate of per-execution device time (no NTFF in this env)."""
    global LAST_NC
    if LAST_NC is None:
        return None
    from concourse.timeline_sim import TimelineSim
    sim = TimelineSim(LAST_NC, trace=False, no_exec=True)
    return int(sim.simulate())


# ---------------- host-side packing ----------------

def pack(names, src_idx, dst_idx, N):
    ntile = (N + NCORE * 128 - 1) // (NCORE * 128)
    nslot = ntile * 128
    nch = 2 * ntile
    deg = np.stack([np.bincount(dst_idx[r], minlength=N) for r in range(R)])
    owner = np.zeros((T, N), np.int32)
    slot = np.zeros((T, N), np.int32)
    NB = NCORE * ntile
    for t in range(T):
        r1, r2 = RELS_OF[t]
        order = np.argsort(-(deg[r1] + deg[r2]), kind='stable')
        bins = [[] for _ in range(NB)]
        load1 = np.zeros(NB, np.int64)
        load2 = np.zeros(NB, np.int64)
        for k in range(0, N, NB):
            nodes = order[k:k + NB]
            seq = range(NB) if (k // NB) % 2 == 0 else range(NB - 1, -1, -1)
            for n, b in zip(nodes, seq):
                bins[b].append(n)
                load1[b] += deg[r1][n]
                load2[b] += deg[r2][n]
        sizes = np.array([len(b) for b in bins])
        for _ in range(400):
            bad = np.where((load1 > CAP) | (load2 > CAP))[0]
            if len(bad) == 0:
                break
            for b in bad:
                while load1[b] > CAP or load2[b] > CAP:
                    nb = max(bins[b], key=lambda n: deg[r1][n] + deg[r2][n])
                    cand = int(np.argmin(load1 + load2 + (sizes >= 128) * (1 << 40)))
                    bins[b].remove(nb)
                    load1[b] -= deg[r1][nb]; load2[b] -= deg[r2][nb]; sizes[b] -= 1
                    bins[cand].append(nb)
                    load1[cand] += deg[r1][nb]; load2[cand] += deg[r2][nb]; sizes[cand] += 1
        assert (load1 <= CAP).all() and (load2 <= CAP).all()
        for b in range(NB):
            c, tl = b % NCORE, b // NCORE
            for p, n in enumerate(bins[b]):
                owner[t][n] = c
                slot[t][n] = tl * 128 + p

    # per-type global row id inside xfull[type]: owner*nslot + slot
    rowid = owner.astype(np.int64) * nslot + slot  # [T,N]

    node_at = np.full((T, NCORE, nslot), -1, np.int64)
    for t in range(T):
        node_at[t, owner[t], slot[t]] = np.arange(N)

    ECH = nch * 128
    srcrowT = np.zeros((R, NCORE, 128, nch), np.int32)
    qtidxT = np.zeros((R, NCORE, 128, nch), np.int32)
    dstoffT = np.full((R, NCORE, 128, nch), 200.0, np.float32)
    for r in range(R):
        st, dt = REL_SRC[r], REL_DST[r]
        s, d = src_idx[r], dst_idx[r]
        ce = owner[dt][d]
        sl = slot[dt][d]
        srow = rowid[st][s].astype(np.int32)
        for c in range(NCORE):
            m = ce == c
            tl = (sl[m] >> 7).astype(np.int64)
            o2 = np.argsort(tl, kind='stable')
            tls = tl[o2]
            cnt = np.bincount(tls, minlength=ntile)
            starts = np.zeros(ntile, np.int64)
            starts[1:] = np.cumsum(cnt)[:-1]
            within = np.arange(len(tls)) - np.repeat(starts, cnt)
            place = tls * CAP + within
            SR = np.zeros(ECH, np.int32)
            QI = np.zeros(ECH, np.int32)
            DO = np.full(ECH, 200.0, np.float32)
            SR[place] = srow[m][o2]
            QI[place] = sl[m][o2]
            DO[place] = (sl[m] & 127)[o2].astype(np.float32)
            srcrowT[r, c] = SR.reshape(nch, 128).T
            qtidxT[r, c] = QI.reshape(nch, 128).T
            dstoffT[r, c] = DO.reshape(nch, 128).T

    cntn = np.zeros((T, N), np.float32)
    for t in range(T):
        for r in RELS_OF[t]:
            cntn[t] += (deg[r] > 0)
    invn = 1.0 / np.maximum(cntn, 1.0)
    invT = np.ones((NCORE, T, 128, ntile), np.float32)
    embidxT = np.zeros((NCORE, 128, T * ntile), np.int32)
    for t in range(T):
        for c in range(NCORE):
            na = node_at[t, c]
            live = na >= 0
            iv = np.ones(nslot, np.float32)
            iv[live] = invn[t][na[live]]
            invT[c, t] = iv.reshape(ntile, 128).T
            er = np.zeros(nslot, np.int32)
            er[live] = names[t][na[live]]
            embidxT[c, :, t * ntile:(t + 1) * ntile] = er.reshape(ntile, 128).T

    return dict(ntile=ntile, nslot=nslot, nch=nch, owner=owner, slot=slot,
                srcrowT=srcrowT, qtidxT=qtidxT, dstoffT=dstoffT,
                invT=invT, embidxT=embidxT, node_at=node_at)


def fold_weights(w):
    KWMSG = np.zeros((L, R, D, 2 * D), np.float32)   # [K | WMSG]
    BIAS2 = np.zeros((L, R, 2 * D), np.float32)      # [0 | MSGB]
    W2 = np.zeros((L, R, D, 132), np.float32)
    B2 = np.zeros((L, R, 132), np.float32)
    for l in range(L):
        for r in range(R):
            st, dt = REL_SRC[r], REL_DST[r]
            ratp = w['rel_att'][l, r] * (w['rel_pri'][l, r] / SQRT_DK)[:, None, None]
            M = np.zeros((D, D), np.float32)
            BD = np.zeros((D, D), np.float32)
            for h in range(H):
                M[h * DK:(h + 1) * DK, h * DK:(h + 1) * DK] = ratp[h].T
                BD[h * DK:(h + 1) * DK, h * DK:(h + 1) * DK] = w['rel_msg'][l, r, h]
            kb = w['k_b'][l, st]
            Ckb = np.zeros((D, H), np.float32)
            for h in range(H):
                Ckb[h * DK:(h + 1) * DK, h] = kb[h * DK:(h + 1) * DK]
            MA = np.concatenate([M, M @ Ckb], axis=1)  # [128,132]
            W2[l, r] = w['q_w'][l, dt] @ MA
            B2[l, r] = w['q_b'][l, dt] @ MA
            KWMSG[l, r, :, :D] = w['k_w'][l, st]
            KWMSG[l, r, :, D:] = w['v_w'][l, st] @ BD
            BIAS2[l, r, D:] = w['v_b'][l, st] @ BD
    alphas = 1.0 / (1.0 + np.exp(-np.asarray(w['skip'], np.float32)))  # [L,T]
    AWF = np.asarray(w['a_w'], np.float32) * alphas[:, :, None, None]
    ABF = np.asarray(w['a_b'], np.float32) * alphas[:, :, None]
    lng = np.asarray(w['ln_g'], np.float32)
    lnb = np.asarray(w['ln_b'], np.float32)
    ln_trivial = bool(np.all(lng == 1.0) and np.all(lnb == 0.0))
    return dict(KWMSG=KWMSG, BIAS2=BIAS2, W2=W2, B2=B2, alphas=alphas,
                AWF=AWF, ABF=ABF, ln_trivial=ln_trivial, lng=lng, lnb=lnb)


# ---------------- numpy model (validation mirror of the device program) ----

def numpy_forward(P, fw, emb, w, N):
    ntile, nslot, nch = P['ntile'], P['nslot'], P['nch']
    # adapt
    xloc = []
    for c in range(NCORE):
        rows = P['embidxT'][c].T.reshape(-1)  # [T*nslot] local row-major
        inp = emb[rows]  # [T*nslot,128]
        h0 = np.zeros((T * nslot, D), np.float32)
        for t in range(T):
            blk = inp[t * nslot:(t + 1) * nslot]
            h0[t * nslot:(t + 1) * nslot] = np.tanh(blk @ w['adapt_w'][t] + w['adapt_b'][t])
        xloc.append(h0)
    # per-type full tensors [NCORE*nslot, D]
    xfull = [np.concatenate([xloc[c][t * nslot:(t + 1) * nslot] for c in range(NCORE)], 0)
             for t in range(T)]
    for l in range(L):
        newloc = []
        for c in range(NCORE):
            out_c = np.zeros((T * nslot, D), np.float32)
            for dt in range(T):
                tacc = np.zeros((nslot, D), np.float32)
                for r in RELS_OF[dt]:
                    st = REL_SRC[r]
                    xl = xloc[c][dt * nslot:(dt + 1) * nslot]
                    qt = xl @ fw['W2'][l, r] + fw['B2'][l, r]  # [nslot,132]
                    sr = P['srcrowT'][r, c].T.reshape(-1)  # [ECH]
                    qi = P['qtidxT'][r, c].T.reshape(-1)
                    do = P['dstoffT'][r, c].T.reshape(-1)
                    X = xfull[st][sr]  # [ECH,128]
                    kemsg = X @ fw['KWMSG'][l, r] + fw['BIAS2'][l, r]  # [ECH,256]
                    ke, msgp = kemsg[:, :D], kemsg[:, D:]
                    QT = qt[qi]
                    att = (ke * QT[:, :D]).reshape(-1, H, DK).sum(-1) + QT[:, D:]
                    A = np.exp(att)  # [ECH,H]
                    mw = msgp * np.repeat(A, DK, axis=1)
                    S = np.zeros((nslot, D), np.float32)
                    ss = np.zeros((nslot, H), np.float32)
                    for tl in range(ntile):
                        sl_ = slice(tl * CAP, (tl + 1) * CAP)
                        mask = do[sl_, None] == np.arange(128)[None, :]  # [CAP,128]
                        S[tl * 128:(tl + 1) * 128] += mask.T @ mw[sl_]
                        ss[tl * 128:(tl + 1) * 128] += mask.T @ A[sl_]
                    iv = P['invT'][c, dt].T.reshape(-1)
                    rr = iv[:, None] / (ss + 1e-20)
                    tacc += S * np.repeat(rr, DK, 1)
                trans = tacc @ fw['AWF'][l, dt] + fw['ABF'][l, dt]
                al = fw['alphas'][l, dt]
                o = trans + xloc[c][dt * nslot:(dt + 1) * nslot] * (1 - al)
                mu = o.mean(-1, keepdims=True)
                var = (o ** 2).mean(-1, keepdims=True) - mu ** 2
                o = fw['lng'][l, dt] * (o - mu) / np.sqrt(var + EPS) + fw['lnb'][l, dt]
                out_c[dt * nslot:(dt + 1) * nslot] = o
            newloc.append(out_c)
        xloc = newloc
        xfull = [np.concatenate([xloc[c][t * nslot:(t + 1) * nslot] for c in range(NCORE)], 0)
                 for t in range(T)]
    return xloc  # per-core local outputs [T*nslot, D]


def unpack_output(P, outs, N):
    nslot = P['nslot']
    res = np.zeros((T, N, D), np.float32)
    for t in range(T):
        ow, sl = P['owner'][t], P['slot'][t]
        allc = np.stack([outs[c][t * nslot:(t + 1) * nslot] for c in range(NCORE)])
        res[t] = allc[ow, sl]
    return res


# ---------------- device program ----------------

def build_nc(P, fw, debug=False):
    ntile, nslot, nch = P['ntile'], P['nslot'], P['nch']
    assert nch % 4 == 0 and ntile % 2 == 0
    GROWS = NCORE * nslot
    alphas = fw['alphas']
    ln_trivial = fw['ln_trivial']
    nc = bacc.Bacc("TRN2", target_bir_lowering=False, debug=False, num_devices=NCORE)

    def din(name, shape, dt=BF16):
        return nc.dram_tensor(name, list(shape), dt, kind="ExternalInput")

    emb_t = din("emb", (P['V'], D))
    embidx_t = din("embidx", (128, T * ntile), I32)
    srcrow_t = din("srcrow", (R * 128, nch), I32)
    qtidx_t = din("qtidx", (R * 128, nch), I32)
    dstoff_t = din("dstoff", (R * 128, nch))
    inv_t = din("invt", (T * 128, ntile), F32)
    KWMSG_t = din("kwmsg", (L * R * 128, 2 * D))
    BIAS2_t = din("bias2", (L * R, 2 * D))
    W2_t = din("w2", (L * R * 128, 132))
    B2_t = din("b2", (L * R, 132))
    ADW_t = din("adw", (T * 128, D))
    ADB_t = din("adb", (T, D))
    AW_t = din("aw", (L * T * 128, D))
    AB_t = din("ab", (L * T, D))
    IOTA_t = din("iota4", (128, 4 * 128))
    IDENT_t = din("ident", (128, 128))
    ONES_t = din("ones", (1, 128))
    if not ln_trivial:
        G_t = din("lng", (L * T * 128, D), F32)
        BB_t = din("lnb", (L * T * 128, D), F32)
    out_t = nc.dram_tensor("outloc", [T * nslot, D], F32, kind="ExternalOutput")

    # per (layer, type) local slabs and replicated slabs
    dbg_kind = "ExternalOutput" if debug else "Internal"
    hloc = [[nc.dram_tensor(f"hloc{l}_{t}", [nslot, D], BF16) for t in range(T)]
            for l in range(L)]
    hlocB = [[nc.dram_tensor(f"hlocB{l}_{t}", [nslot, D], BF16, kind=dbg_kind) for t in range(T)]
             for l in range(L)]
    xfull = [[nc.dram_tensor(f"xfull{l}_{t}", [GROWS, D], BF16, addr_space="Shared")
              for t in range(T)] for l in range(L)]
    qtt = [[[nc.dram_tensor(f"qtt{l}_{t}_{i}", [nslot, 132], BF16, kind=dbg_kind)
             for i in range(2)]
            for t in range(T)] for l in range(L)]

    from contextlib import ExitStack
    with tile.TileContext(nc) as tc, ExitStack() as es:
        cp = es.enter_context(tc.tile_pool(name="consts", bufs=1))
        ident = cp.tile([128, 128], BF16); nc.sync.dma_start(out=ident[:], in_=IDENT_t[:, :])
        iota4 = cp.tile([128, 4, 128], BF16); nc.sync.dma_start(out=iota4[:], in_=IOTA_t[:, :])
        ones = cp.tile([1, 128], BF16); nc.sync.dma_start(out=ones[:], in_=ONES_t[:, :])
        epst = cp.tile([128, 1], F32); nc.vector.memset(epst[:], EPS)

        wp = es.enter_context(tc.tile_pool(name="wts", bufs=2))
        ip = es.enter_context(tc.tile_pool(name="idx", bufs=2))
        gp = es.enter_context(tc.tile_pool(name="gath", bufs=2))
        pp = es.enter_context(tc.tile_pool(name="ps", bufs=1, space="PSUM"))
        pp2 = es.enter_context(tc.tile_pool(name="ps2", bufs=2, space="PSUM"))
        ppk = es.enter_context(tc.tile_pool(name="psk", bufs=2, space="PSUM"))
        sp = es.enter_context(tc.tile_pool(name="work", bufs=3))
        ap_ = es.enter_context(tc.tile_pool(name="acc", bufs=1))

        def tr(dst_psum, src):
            nc.tensor.transpose(out=dst_psum, in_=src, identity=ident[:])

        AG = mybir.AluOpType.bypass
        MUL = mybir.AluOpType.mult
        ADD = mybir.AluOpType.add
        SUB = mybir.AluOpType.subtract
        EQ = mybir.AluOpType.is_equal
        AF = mybir.ActivationFunctionType

        # ---------------- adapt phase (per type, then allgather) ----------
        embidx = ip.tile([128, T * ntile], I32, tag="embidx")
        nc.sync.dma_start(out=embidx[:], in_=embidx_t[:, :])
        for t in range(T):
            adw = wp.tile([128, D], BF16, tag="adw")
            nc.sync.dma_start(out=adw[:], in_=ADW_t[t * 128:(t + 1) * 128, :])
            adb = wp.tile([1, D], BF16, tag="adb")
            nc.sync.dma_start(out=adb[:], in_=ADB_t[t:t + 1, :])
            for base in range(0, ntile, 8):
                k = min(8, ntile - base)
                xg = gp.tile([128, 8, D], BF16, tag="exg")
                for j in range(k):
                    nc.gpsimd.indirect_dma_start(
                        out=xg[:, j, :], out_offset=None, in_=emb_t[:, :],
                        in_offset=bass.IndirectOffsetOnAxis(
                            ap=embidx[:, t * ntile + base + j:t * ntile + base + j + 1], axis=0))
                hs = sp.tile([128, 8, D], BF16, tag="hs8")
                for j in range(k):
                    tp = pp2.tile([128, 512], BF16, tag="tp4")
                    tr(tp[:, 0:128], xg[:, j, :])
                    gT = sp.tile([128, 128], BF16, tag="gT")
                    nc.vector.tensor_copy(out=gT[:], in_=tp[:, 0:128])
                    h0 = ppk.tile([128, 4, 2 * D], F32, tag="kemsg")
                    nc.tensor.matmul(out=h0[:, 0, 0:D], lhsT=gT[:], rhs=adw[:], start=True, stop=False)
                    nc.tensor.matmul(out=h0[:, 0, 0:D], lhsT=ones[:], rhs=adb[:], start=False, stop=True)
                    nc.scalar.activation(out=hs[:, j, :], in_=h0[:, 0, 0:D], func=AF.Tanh)
                rows = hloc[0][t][base * 128:(base + k) * 128, :]
                nc.sync.dma_start(out=rows.rearrange("(j p) f -> p j f", p=128),
                                  in_=hs[:, 0:k, :])
                rowsB = hlocB[0][t][base * 128:(base + k) * 128, :]
                nc.sync.dma_start(out=rowsB.rearrange("(j p) f -> p j f", p=128),
                                  in_=hs[:, 0:k, :])
            nc.gpsimd.collective_compute(
                "AllGather", AG, replica_groups=[list(range(NCORE))],
                ins=[hloc[0][t].ap().opt()], outs=[xfull[0][t].ap().opt()])

        # ---------------- qt phase --------------------------------------
        def qt_phase(l, dt):
            w2s, b2s = [], []
            for ri, r in enumerate(RELS_OF[dt]):
                w2 = wp.tile([128, 132], BF16, tag=f"w2{ri}")
                nc.sync.dma_start(out=w2[:], in_=W2_t[(l * R + r) * 128:(l * R + r + 1) * 128, :])
                b2 = wp.tile([1, 132], BF16, tag=f"b2{ri}")
                nc.sync.dma_start(out=b2[:], in_=B2_t[l * R + r:l * R + r + 1, :])
                w2s.append(w2); b2s.append(b2)
            for base in range(0, ntile, 8):
                k = min(8, ntile - base)
                xc8 = sp.tile([128, 8, D], BF16, tag="xc8")
                rows = hlocB[l][dt][base * 128:(base + k) * 128, :]
                nc.sync.dma_start(out=xc8[:, 0:k, :],
                                  in_=rows.rearrange("(j p) f -> p j f", p=128))
                qs8 = [sp.tile([128, 8, 132], BF16, tag=f"qs8_{i}", name=f"qs8_{i}") for i in range(2)]
                for j in range(k):
                    tp = pp2.tile([128, 512], BF16, tag="tp4")
                    tr(tp[:, 0:128], xc8[:, j, :])
                    xcT = sp.tile([128, 128], BF16, tag="xcT")
                    nc.scalar.activation(out=xcT[:], in_=tp[:, 0:128], func=AF.Copy)
                    for ri in range(2):
                        qts = ppk.tile([128, 4, 2 * D], F32, tag="kemsg")
                        nc.tensor.matmul(out=qts[:, 0, 0:132], lhsT=xcT[:], rhs=w2s[ri][:], start=True, stop=False)
                        nc.tensor.matmul(out=qts[:, 0, 0:132], lhsT=ones[:], rhs=b2s[ri][:], start=False, stop=True)
                        nc.vector.tensor_copy(out=qs8[ri][:, j, :], in_=qts[:, 0, 0:132])
                for ri in range(2):
                    rows = qtt[l][dt][ri][base * 128:(base + k) * 128, :]
                    nc.sync.dma_start(out=rows.rearrange("(j p) f -> p j f", p=128),
                                      in_=qs8[ri][:, 0:k, :])

        # ---------------- edge + finish phase ----------------------------
        def edge_finish(l, dt):
            rels = RELS_OF[dt]
            kw_t, bia_t, srct, qit, dot = [], [], [], [], []
            for ri, r in enumerate(rels):
                kwm = wp.tile([128, 2 * D], BF16, tag=f"kwm{ri}")
                nc.sync.dma_start(out=kwm[:], in_=KWMSG_t[(l * R + r) * 128:(l * R + r + 1) * 128, :])
                bia = wp.tile([1, 2 * D], BF16, tag=f"bia{ri}")
                nc.sync.dma_start(out=bia[:], in_=BIAS2_t[l * R + r:l * R + r + 1, :])
                st_ = ip.tile([128, nch], I32, tag=f"srct{ri}")
                nc.sync.dma_start(out=st_[:], in_=srcrow_t[r * 128:(r + 1) * 128, :])
                qi_ = ip.tile([128, nch], I32, tag=f"qit{ri}")
                nc.sync.dma_start(out=qi_[:], in_=qtidx_t[r * 128:(r + 1) * 128, :])
                do_ = ip.tile([128, nch, 1], BF16, tag=f"dot{ri}")
                nc.sync.dma_start(out=do_[:], in_=dstoff_t[r * 128:(r + 1) * 128, :])
                kw_t.append(kwm); bia_t.append(bia); srct.append(st_); qit.append(qi_); dot.append(do_)
            ivt = ip.tile([128, ntile, 1, 1], F32, tag="ivt")
            nc.sync.dma_start(out=ivt[:], in_=inv_t[dt * 128:(dt + 1) * 128, :])
            tacc = ap_.tile([128, ntile, 128], BF16, tag="tacc")

            for base in range(0, nch, 8):
                k = min(8, nch - base)
                XG8, QT8 = [], []
                for ri, r in enumerate(rels):
                    st = REL_SRC[r]
                    xg = gp.tile([128, 8, D], BF16, tag=f"XG{ri}")
                    q8 = gp.tile([128, 8, 132], BF16, tag=f"QT{ri}")
                    for j in range(k):
                        nc.gpsimd.indirect_dma_start(
                            out=xg[:, j, :], out_offset=None, in_=xfull[l][st][:, :],
                            in_offset=bass.IndirectOffsetOnAxis(
                                ap=srct[ri][:, base + j:base + j + 1], axis=0))
                        nc.gpsimd.indirect_dma_start(
                            out=q8[:, j, :], out_offset=None, in_=qtt[l][dt][ri][:, :],
                            in_offset=bass.IndirectOffsetOnAxis(
                                ap=qit[ri][:, base + j:base + j + 1], axis=0))
                    XG8.append(xg); QT8.append(q8)
                for half in range(k // 4):
                    c0 = base + half * 4      # first chunk of this 4-group
                    g2 = c0 // 2              # first tile of the pair
                    Sps = pp.tile([128, 2, 2, 128], F32, tag="Sps")
                    ssp = pp.tile([128, 2, 2, 4], F32, tag="ssp")
                    for ri in range(2):
                        tp4 = pp2.tile([128, 512], BF16, tag="tp4")
                        for c in range(4):
                            tr(tp4[:, c * 128:(c + 1) * 128], XG8[ri][:, half * 4 + c, :])
                        XT4 = sp.tile([128, 512], BF16, tag="XT4")
                        nc.scalar.activation(out=XT4[:], in_=tp4[:], func=AF.Copy)
                        kemsg = ppk.tile([128, 4, 2 * D], F32, tag="kemsg")
                        for c in range(4):
                            nc.tensor.matmul(out=kemsg[:, c, :], lhsT=XT4[:, c * 128:(c + 1) * 128],
                                             rhs=kw_t[ri][:], start=True, stop=False)
                            nc.tensor.matmul(out=kemsg[:, c, :], lhsT=ones[:],
                                             rhs=bia_t[ri][:], start=False, stop=True)
                        msk4 = sp.tile([128, 4, 128], BF16, tag="msk4")
                        nc.vector.tensor_tensor(
                            out=msk4[:], in0=dot[ri][:, c0:c0 + 4, :].to_broadcast([128, 4, 128]),
                            in1=iota4[:], op=EQ)
                        P4 = sp.tile([128, 4, 4, 32], BF16, tag="P4")
                        nc.vector.tensor_tensor(out=P4[:], in0=kemsg[:, :, 0:D],
                                                in1=QT8[ri][:, half * 4:half * 4 + 4, 0:D],
                                                op=MUL)
                        attE = sp.tile([128, 4, 4], F32, tag="attE")
                        nc.vector.tensor_reduce(out=attE[:], in_=P4[:],
                                                axis=mybir.AxisListType.X, op=ADD)
                        nc.vector.tensor_tensor(out=attE[:], in0=attE[:],
                                                in1=QT8[ri][:, half * 4:half * 4 + 4, D:132],
                                                op=ADD)
                        A4 = sp.tile([128, 4, 4, 1], F32, tag="A4")
                        nc.scalar.activation(out=A4[:], in_=attE[:], func=AF.Exp)
                        mw4 = sp.tile([128, 4, 132], BF16, tag="mw4")
                        nc.vector.tensor_tensor(out=mw4[:, :, 0:D], in0=kemsg[:, :, D:2 * D],
                                                in1=A4[:].to_broadcast([128, 4, 4, 32]),
                                                op=MUL)
                        nc.vector.tensor_copy(out=mw4[:, :, D:132], in_=A4[:])
                        for c in range(4):
                            tl2, cc = c // 2, c % 2
                            nc.tensor.matmul(out=Sps[:, tl2, ri, :], lhsT=msk4[:, c, :],
                                             rhs=mw4[:, c, 0:D],
                                             start=(cc == 0), stop=(cc == 1),
                                             skip_group_check=True)
                            nc.tensor.matmul(out=ssp[:, tl2, ri, :], lhsT=msk4[:, c, :],
                                             rhs=mw4[:, c, D:132],
                                             start=(cc == 0), stop=(cc == 1),
                                             skip_group_check=True)
                    # epilogue for the 2-tile pair, both relations
                    srec = sp.tile([128, 2, 2, 4], F32, tag="srec")
                    nc.vector.tensor_scalar(out=srec[:], in0=ssp[:], scalar1=1e-20,
                                            scalar2=None, op0=ADD)
                    nc.vector.reciprocal(out=srec[:], in_=srec[:])
                    nc.vector.tensor_tensor(out=srec[:], in0=srec[:],
                                            in1=ivt[:, g2:g2 + 2, :, :].to_broadcast([128, 2, 2, 4]),
                                            op=MUL)
                    u = sp.tile([128, 2, 2, 128], BF16, tag="u")
                    nc.vector.tensor_tensor(out=u[:], in0=Sps[:],
                                            in1=srec[:].to_broadcast([128, 2, 2, 4, 32]),
                                            op=MUL)
                    nc.vector.scalar_tensor_tensor(
                        out=tacc[:, g2:g2 + 2, :], in0=u[:, :, 0, :], scalar=0.0,
                        in1=u[:, :, 1, :], op0=ADD, op1=ADD)

            # ---- finish ----
            aw = wp.tile([128, D], BF16, tag="aw")
            nc.sync.dma_start(out=aw[:], in_=AW_t[(l * T + dt) * 128:(l * T + dt + 1) * 128, :])
            ab = wp.tile([1, D], BF16, tag="abb")
            nc.sync.dma_start(out=ab[:], in_=AB_t[l * T + dt:l * T + dt + 1, :])
            if not ln_trivial:
                gt = wp.tile([128, D], F32, tag="gt")
                nc.sync.dma_start(out=gt[:], in_=G_t[(l * T + dt) * 128:(l * T + dt + 1) * 128, :])
                bt = wp.tile([128, D], F32, tag="bt")
                nc.sync.dma_start(out=bt[:], in_=BB_t[(l * T + dt) * 128:(l * T + dt + 1) * 128, :])
            al = float(alphas[l, dt])
            last = (l == L - 1)
            odt = F32 if last else BF16
            for base in range(0, ntile, 8):
                k = min(8, ntile - base)
                xc8 = sp.tile([128, 8, D], BF16, tag="xc8")
                rows = hlocB[l][dt][base * 128:(base + k) * 128, :]
                nc.sync.dma_start(out=xc8[:, 0:k, :],
                                  in_=rows.rearrange("(j p) f -> p j f", p=128))
                o8 = sp.tile([128, 8, D], BF16, tag="o8")
                oo8 = sp.tile([128, 8, D], odt, tag="oo8")
                musum = sp.tile([128, 8, 1], F32, tag="musum")
                sqsum = sp.tile([128, 8, 1], F32, tag="sqsum")
                sqd = sp.tile([128, D], BF16, tag="sqd")
                for j in range(k):
                    tl = base + j
                    tp = pp2.tile([128, 512], BF16, tag="tp4")
                    tr(tp[:, 0:128], tacc[:, tl, :])
                    ttT = sp.tile([128, 128], BF16, tag="ttT")
                    nc.scalar.activation(out=ttT[:], in_=tp[:, 0:128], func=AF.Copy)
                    trp = ppk.tile([128, 4, 2 * D], F32, tag="kemsg")
                    nc.tensor.matmul(out=trp[:, 0, 0:D], lhsT=ttT[:], rhs=aw[:], start=True, stop=False)
                    nc.tensor.matmul(out=trp[:, 0, 0:D], lhsT=ones[:], rhs=ab[:], start=False, stop=True)
                    nc.vector.scalar_tensor_tensor(
                        out=o8[:, j, :], in0=xc8[:, j, :], scalar=1.0 - al,
                        in1=trp[:, 0, 0:D], op0=MUL, op1=ADD,
                        accum_out=musum[:, j, :])
                    nc.scalar.activation(out=sqd[:], in_=o8[:, j, :], func=AF.Square,
                                         accum_out=sqsum[:, j, :])
                mu8 = sp.tile([128, 8, 1], F32, tag="mu8")
                nc.vector.tensor_scalar(out=mu8[:, 0:k, :], in0=musum[:, 0:k, :],
                                        scalar1=1.0 / D, scalar2=None, op0=MUL)
                m2 = sp.tile([128, 8, 1], F32, tag="m2")
                nc.vector.tensor_tensor(out=m2[:, 0:k, :], in0=mu8[:, 0:k, :],
                                        in1=mu8[:, 0:k, :], op=MUL)
                var8 = sp.tile([128, 8, 1], F32, tag="var8")
                nc.vector.scalar_tensor_tensor(
                    out=var8[:, 0:k, :], in0=sqsum[:, 0:k, :], scalar=1.0 / D,
                    in1=m2[:, 0:k, :], op0=MUL, op1=SUB)
                nc.scalar.activation(out=var8[:, 0:k, :], in_=var8[:, 0:k, :],
                                     func=AF.Sqrt, bias=epst[:, 0:1])
                nc.vector.reciprocal(out=var8[:, 0:k, :], in_=var8[:, 0:k, :])
                for j in range(k):
                    nc.vector.tensor_scalar(out=oo8[:, j, :], in0=o8[:, j, :],
                                            scalar1=mu8[:, j, :], scalar2=var8[:, j, :],
                                            op0=SUB, op1=MUL)
                    if not ln_trivial:
                        nc.vector.tensor_tensor(out=oo8[:, j, :], in0=oo8[:, j, :],
                                                in1=gt[:], op=MUL)
                        nc.vector.tensor_tensor(out=oo8[:, j, :], in0=oo8[:, j, :],
                                                in1=bt[:], op=ADD)
                if last:
                    rows = out_t[dt * nslot + base * 128:dt * nslot + (base + k) * 128, :]
                    nc.sync.dma_start(out=rows.rearrange("(j p) f -> p j f", p=128),
                                      in_=oo8[:, 0:k, :])
                else:
                    rows = hloc[l + 1][dt][base * 128:(base + k) * 128, :]
                    nc.sync.dma_start(out=rows.rearrange("(j p) f -> p j f", p=128),
                                      in_=oo8[:, 0:k, :])
                    rowsB = hlocB[l + 1][dt][base * 128:(base + k) * 128, :]
                    nc.sync.dma_start(out=rowsB.rearrange("(j p) f -> p j f", p=128),
                                      in_=oo8[:, 0:k, :])
            if not last:
                nc.gpsimd.collective_compute(
                    "AllGather", AG, replica_groups=[list(range(NCORE))],
                    ins=[hloc[l + 1][dt].ap().opt()], outs=[xfull[l + 1][dt].ap().opt()])

        # schedule: qt for layer 0 first (overlaps first allgathers), then
        # per-type edge+finish with per-type allgathers interleaved.
        # no_sync_barrier keeps the scheduler from hoisting collective-
        # dependent edge work ahead of ready work (head-of-line blocking
        # on the in-order engine queues).
        for dt in ORD[0]:
            qt_phase(0, dt)
        for i, dt in enumerate(ORD[0]):
            tc.no_sync_barrier()
            edge_finish(0, dt)
            qt_phase(1, dt)
        for dt in ORD[1]:
            tc.no_sync_barrier()
            edge_finish(1, dt)

    nc.compile()
    return nc


def _in_maps(P, fw, inputs, emb):
    ntile, nslot, nch = P['ntile'], P['nslot'], P['nch']
    iota4 = np.tile(np.arange(128, dtype=np.float32), (128, 4)).astype(BF)
    ident = np.eye(128, dtype=np.float32).astype(BF)
    onesr = np.ones((1, 128), BF)
    com = dict(
        emb=emb.astype(BF),
        kwmsg=fw['KWMSG'].reshape(L * R * 128, 2 * D).astype(BF),
        bias2=fw['BIAS2'].reshape(L * R, 2 * D).astype(BF),
        w2=fw['W2'].reshape(L * R * 128, 132).astype(BF),
        b2=fw['B2'].reshape(L * R, 132).astype(BF),
        adw=np.ascontiguousarray(inputs['adapt_w'], np.float32).reshape(T * 128, D).astype(BF),
        adb=np.asarray(inputs['adapt_b'], np.float32).astype(BF),
        aw=fw['AWF'].reshape(L * T * 128, D).astype(BF),
        ab=fw['ABF'].reshape(L * T, D).astype(BF),
        iota4=iota4, ident=ident, ones=onesr,
    )
    if not fw['ln_trivial']:
        com['lng'] = np.repeat(fw['lng'].reshape(L * T, 1, D), 128, 1).reshape(L * T * 128, D)
        com['lnb'] = np.repeat(fw['lnb'].reshape(L * T, 1, D), 128, 1).reshape(L * T * 128, D)
    in_maps = []
    for c in range(NCORE):
        m = dict(com)
        m['embidx'] = P['embidxT'][c]
        m['srcrow'] = P['srcrowT'][:, c].reshape(R * 128, nch)
        m['qtidx'] = P['qtidxT'][:, c].reshape(R * 128, nch)
        m['dstoff'] = P['dstoffT'][:, c].reshape(R * 128, nch).astype(BF)
        m['invt'] = P['invT'][c].reshape(T * 128, ntile)
        in_maps.append(m)
    return in_maps


def kernel(**inputs):
    global LAST_EXEC_NS, LAST_NC
    names = np.asarray(inputs['names'])
    src_idx = np.asarray(inputs['src_idx'])
    dst_idx = np.asarray(inputs['dst_idx'])
    emb = np.asarray(inputs['node_emb'], np.float32)
    N = names.shape[1]
    V = emb.shape[0]
    P = pack(names, src_idx, dst_idx, N)
    P['V'] = V
    fw = fold_weights(inputs)
    nc = build_nc(P, fw)
    in_maps = _in_maps(P, fw, inputs, emb)
    import os
    res = run_bass_kernel_spmd(nc, in_maps, core_ids=list(range(NCORE)))
    LAST_EXEC_NS = res.exec_time_ns
    LAST_NC = nc
    outs = [res.results[c]["outloc"] for c in range(NCORE)]
    return unpack_output(P, outs, N)
